# revision 25
# baseline (speedup 1.0000x reference)
"""Trainium2 Bass kernel for nn_BlockV2 (conv -> LN -> minGRU -> MLP x4).

Strategy: data-parallel over batch (B=8 -> 8 cores). Per core, activations
are kept in [D_partitions, T_free] layout and streamed through each layer in
chunks of 512 tokens; inter-layer activations ping-pong through DRAM.
The minGRU recurrence h_t = c_t*h_{t-1} + v_t runs on the VectorE
tensor_tensor_scan instruction (fp32 state), chained across chunks.

v2 changes vs baseline:
- LN statistics matmuls run in bf16 (the fp32 ones were LOW_HIGH two-pass,
  ~4x the cost); the bf16 stat input copies are made on the idle GpSimd
  engine, and the per-token mean/rstd broadcasts use gpsimd
  partition_broadcast instead of TensorE ones-matmuls.
- f_w@conv_pw_w is fused host-side (FW'), so the mid-layer GRU kh matmul
  consumes the depthwise-conv output y directly - the pointwise conv output
  cv is only needed (in bf16) for the residual add.
- conv_dw runs fully in bf16 on VectorE (2x rate); the MLP output tile m is
  stored bf16 (it only feeds conv_dw).
- layer-0 chunks (TensorE-light, VectorE-heavy) are interleaved into the
  first mid layer's chunk stream so TensorE never starves during the ramp.
- PSUM evacuations are split between ScalarE and VectorE.
"""
import sys

sys.path.insert(0, "/opt/trn_rl_repo")

from contextlib import ExitStack

import numpy as np
import ml_dtypes

import concourse.bass as bass
import concourse.tile as tile
from concourse import bacc, mybir

f32 = mybir.dt.float32
bf16 = mybir.dt.bfloat16
Alu = mybir.AluOpType
Act = mybir.ActivationFunctionType
BF = ml_dtypes.bfloat16

B, D, L, K, H = 8, 512, 4, 4, 2048
N_CORES = 8
LN_EPS = 1e-5
P = 128


def build_nc(T=4096, CH=512, has_lnb=False, debug_outs=False, use_gpsimd=False):
    NCH = T // CH
    DT = D // P      # 4 d-tiles
    HT = H // P      # 16 h-tiles
    E2 = 2 * D
    MT2 = E2 // P    # 8 m-tiles of the kh matmul

    nc = bacc.Bacc("TRN2", target_bir_lowering=False, debug=False)

    xT = nc.dram_tensor("xT", [D, T + 3], bf16, kind="ExternalInput")
    fwT = nc.dram_tensor("fwT", [L, D, E2], bf16, kind="ExternalInput")
    pwT = nc.dram_tensor("pwT", [L, D, D], bf16, kind="ExternalInput")
    w1T = nc.dram_tensor("w1T", [L, D, H], bf16, kind="ExternalInput")
    w2T = nc.dram_tensor("w2T", [L, H, D], bf16, kind="ExternalInput")
    dwDg = nc.dram_tensor("dwDg", [L, P, DT * K * P], bf16, kind="ExternalInput")
    dwb = nc.dram_tensor("dwb", [L, D], f32, kind="ExternalInput")
    pwb = nc.dram_tensor("pwb", [L, D], f32, kind="ExternalInput")
    kbv = nc.dram_tensor("kbv", [L, E2], f32, kind="ExternalInput")
    b1v = nc.dram_tensor("b1v", [L, H], f32, kind="ExternalInput")
    b2v = nc.dram_tensor("b2v", [L, D], f32, kind="ExternalInput")
    lng = nc.dram_tensor("lng", [L + 1, D], f32, kind="ExternalInput")
    lnb = nc.dram_tensor("lnb", [L + 1, D], f32, kind="ExternalInput")
    out_t = nc.dram_tensor("out", [D, T], f32, kind="ExternalOutput")
    if debug_outs:
        xs = [nc.dram_tensor(f"xs{i}", [D, T], f32, kind="ExternalOutput")
              for i in range(L)]
        dbg = {
            "mu": nc.dram_tensor("dbg_mu", [1, 512], f32, kind="ExternalOutput"),
            "var": nc.dram_tensor("dbg_var", [1, 512], f32, kind="ExternalOutput"),
            "rstd": nc.dram_tensor("dbg_rstd", [1, 512], f32, kind="ExternalOutput"),
            "a": nc.dram_tensor("dbg_a", [D, 512], f32, kind="ExternalOutput"),
        }
    else:
        dbg = None
        xs = [nc.dram_tensor(f"xs{i}", [D, T], f32) for i in range(2)]

    def dram3(tensor, c, width):
        return tensor.ap().rearrange("(dt p) t -> p dt t", p=P)[:, :, c * CH: c * CH + width]

    with tile.TileContext(nc) as tc, ExitStack() as ctx:
        sing = ctx.enter_context(tc.tile_pool(name="sing", bufs=1))
        wpool = ctx.enter_context(tc.tile_pool(name="w", bufs=1))
        wpool2 = ctx.enter_context(tc.tile_pool(name="w2", bufs=2))
        big = ctx.enter_context(tc.tile_pool(name="big", bufs=7))
        small = ctx.enter_context(tc.tile_pool(name="small", bufs=8))
        mp = ctx.enter_context(tc.tile_pool(name="mp", bufs=2))
        hidp = ctx.enter_context(tc.tile_pool(name="hid", bufs=1))
        statp = ctx.enter_context(tc.tile_pool(name="stat", bufs=5))
        hcp = ctx.enter_context(tc.tile_pool(name="hc", bufs=2))
        zsp = ctx.enter_context(tc.tile_pool(name="zs", bufs=8))
        dwp = ctx.enter_context(tc.tile_pool(name="dwp", bufs=2))
        psmm = ctx.enter_context(tc.tile_pool(name="psmm", bufs=5, space="PSUM"))
        psst = ctx.enter_context(tc.tile_pool(name="psst", bufs=2, space="PSUM"))
        psbc = ctx.enter_context(tc.tile_pool(name="psbc", bufs=1, space="PSUM"))


        ones_col = sing.tile([P, 1], bf16)
        nc.vector.memset(ones_col, 1.0)
        ones_row = sing.tile([1, P], bf16)
        nc.vector.memset(ones_row, 1.0)
        eps1 = sing.tile([1, 1], f32)
        nc.vector.memset(eps1, LN_EPS)
        zero1 = sing.tile([1, 1], f32)
        nc.vector.memset(zero1, 0.0)
        nhalf_col = sing.tile([P, 1], f32)
        nc.vector.memset(nhalf_col, -0.5)
        zero_col = sing.tile([P, 1], f32)
        nc.vector.memset(zero_col, 0.0)
        dwb_sb = sing.tile([P, L * DT], f32)
        nc.sync.dma_start(out=dwb_sb, in_=dwb.ap().rearrange("l (dt p) -> p (l dt)", p=P))
        pwb_sb = sing.tile([P, L * DT], f32)
        nc.sync.dma_start(out=pwb_sb, in_=pwb.ap().rearrange("l (dt p) -> p (l dt)", p=P))
        kb_sb = sing.tile([P, L * MT2], f32)
        nc.sync.dma_start(out=kb_sb, in_=kbv.ap().rearrange("l (mt p) -> p (l mt)", p=P))
        b1_sb = sing.tile([P, L * HT], f32)
        nc.sync.dma_start(out=b1_sb, in_=b1v.ap().rearrange("l (ht p) -> p (l ht)", p=P))
        b2_sb = sing.tile([P, L * DT], f32)
        nc.sync.dma_start(out=b2_sb, in_=b2v.ap().rearrange("l (dt p) -> p (l dt)", p=P))
        lng_sb = sing.tile([P, (L + 1) * DT], f32)
        nc.sync.dma_start(out=lng_sb, in_=lng.ap().rearrange("l (dt p) -> p (l dt)", p=P))
        lnb_sb = sing.tile([P, (L + 1) * DT], f32)
        nc.sync.dma_start(out=lnb_sb, in_=lnb.ap().rearrange("l (dt p) -> p (l dt)", p=P))

        def load_w(kind, dram, l, shape, pool=wpool):
            t = pool.tile(shape, bf16, tag=kind, name=f"{kind}{l}")
            nc.sync.dma_start(out=t, in_=dram.ap()[l].rearrange("(kt p) e -> p kt e", p=P))
            return t

        def load_dwd(l):
            t = dwp.tile([P, DT, K, P], bf16, tag="dwd", name=f"dwd{l}")
            nc.sync.dma_start(out=t, in_=dwDg.ap()[l].rearrange("p (dt k c) -> p dt k c", dt=DT, k=K))
            return t

        # ---------- LN pieces ----------
        # The residual-stream LN inputs sit on a ~0.5 DC baseline with tiny
        # per-token variance (down to ~1e-5 at the last layer), so the stats
        # are computed on x' = x - shift (exact in fp32, then bf16): the
        # E[x'^2] - mu'^2 cancellation is benign once the DC is removed.
        def ln_stats(x_tile, shift):
            """shifted bf16 copy + xsq + S'/Q' bf16 matmuls. Returns psum tiles."""
            x_bf = small.tile([P, DT, CH], bf16, tag="small", name="x_bf")
            for d in range(DT):
                if d % 2 == 0:
                    sb = nhalf_col if shift == 0.5 else zero_col
                    nc.scalar.activation(out=x_bf[:, d, :], in_=x_tile[:, d, :],
                                         func=Act.Identity, bias=sb[:, :], scale=1.0)
                else:
                    nc.vector.tensor_scalar(
                        out=x_bf[:, d, :], in0=x_tile[:, d, :],
                        scalar1=-shift, scalar2=None, op0=Alu.add)
            xsq = small.tile([P, DT, CH], bf16, tag="small", name="xsq")
            for d in range(DT):
                nc.vector.tensor_mul(xsq[:, d, :], x_bf[:, d, :], x_bf[:, d, :])
            S_ps = psst.tile([1, CH], f32, tag="ps_stat", name="S_ps")
            for kt in range(DT):
                nc.tensor.matmul(S_ps[:, :], ones_col[:, :], x_bf[:, kt, :],
                                 start=(kt == 0), stop=(kt == DT - 1))
            Q_ps = psst.tile([1, CH], f32, tag="ps_stat", name="Q_ps")
            for kt in range(DT):
                nc.tensor.matmul(Q_ps[:, :], ones_col[:, :], xsq[:, kt, :],
                                 start=(kt == 0), stop=(kt == DT - 1))
            return S_ps, Q_ps

        def ln_apply(x_tile, S_ps, Q_ps, slot, out_bf16, shift, dump=None):
            """stat algebra + gpsimd broadcasts + center-in-place + scale.
            mu' = mean(x - shift); center computes (x - shift) - mu'."""
            mu = statp.tile([1, CH], f32, tag="stat", name="mu")
            nc.scalar.activation(out=mu[:, :], in_=S_ps[:, :], func=Act.Identity,
                                 bias=0.0, scale=1.0 / D)
            musq = statp.tile([1, CH], f32, tag="stat", name="musq")
            nc.scalar.activation(out=musq[:, :], in_=mu[:, :], func=Act.Square,
                                 bias=0.0, scale=1.0)
            var = statp.tile([1, CH], f32, tag="stat", name="var")
            nc.vector.scalar_tensor_tensor(
                var[:, :], Q_ps[:, :], 1.0 / D, musq[:, :], Alu.mult, Alu.subtract)
            lnv = statp.tile([1, CH], f32, tag="stat", name="lnv")
            nc.scalar.activation(out=lnv[:, :], in_=var[:, :], func=Act.Ln,
                                 bias=eps1[:, :], scale=1.0)
            rstd = statp.tile([1, CH], f32, tag="stat", name="rstd")
            nc.scalar.activation(out=rstd[:, :], in_=lnv[:, :], func=Act.Exp, scale=-0.5)
            mu_bf = statp.tile([1, CH], bf16, tag="stat", name="mu_bf")
            nc.vector.tensor_copy(out=mu_bf, in_=mu)
            rstd_bf = statp.tile([1, CH], bf16, tag="stat", name="rstd_bf")
            nc.vector.tensor_copy(out=rstd_bf, in_=rstd)
            bc = psbc.tile([P, CH], f32, tag="ps_bc", name="bc")
            nc.tensor.matmul(bc[:, :], ones_row[:, :], mu_bf[:, :], start=True, stop=True)
            for d in range(DT):
                nc.vector.scalar_tensor_tensor(
                    x_tile[:, d, :], x_tile[:, d, :], -shift, bc[:, :],
                    Alu.add, Alu.subtract)
            nc.tensor.matmul(bc[:, :], ones_row[:, :], rstd_bf[:, :], start=True, stop=True)
            if out_bf16:
                a_t = small.tile([P, DT, CH], bf16, tag="small", name="a_t")
            else:
                a_t = big.tile([P, DT, CH], f32, tag="big", name="a_t")
            for d in range(DT):
                nc.vector.scalar_tensor_tensor(
                    a_t[:, d, :], x_tile[:, d, :], lng_sb[:, slot * DT + d: slot * DT + d + 1],
                    bc[:, :], Alu.mult, Alu.mult)
            if dump is not None:
                nc.sync.dma_start(out=dump["mu"].ap(), in_=mu)
                nc.sync.dma_start(out=dump["var"].ap(), in_=var)
                nc.sync.dma_start(out=dump["rstd"].ap(), in_=rstd)
                a_f32 = big.tile([P, DT, CH], f32, tag="big", name="a_f32")
                for d in range(DT):
                    nc.vector.tensor_copy(out=a_f32[:, d, :], in_=a_t[:, d, :])
                nc.sync.dma_start(out=dump["a"].ap().rearrange("(dt p) t -> p dt t", p=P), in_=a_f32)
            if has_lnb:
                for d in range(DT):
                    nc.vector.tensor_scalar(
                        out=a_t[:, d, :], in0=a_t[:, d, :],
                        scalar1=lnb_sb[:, slot * DT + d: slot * DT + d + 1], scalar2=None,
                        op0=Alu.add)
            return a_t

        def mlp_chunk(a_t, l, w1_sb, w2_sb, out_tile, out_off, out_f32_scalar_evac):
            hid = hidp.tile([P, HT, CH], bf16, tag="hid", name="hid")
            for mt in range(HT):
                ps = psmm.tile([P, CH], f32, tag="mm", name="ps1")
                for kt in range(DT):
                    nc.tensor.matmul(ps[:, :], w1_sb[:, kt, bass.ts(mt, P)], a_t[:, kt, :],
                                     start=(kt == 0), stop=(kt == DT - 1))
                RELU_SPLIT = True
                if RELU_SPLIT and mt % 2 == 1:
                    nc.vector.tensor_scalar(
                        out=hid[:, mt, :], in0=ps[:, :],
                        scalar1=b1_sb[:, l * HT + mt: l * HT + mt + 1], scalar2=0.0,
                        op0=Alu.add, op1=Alu.max)
                else:
                    nc.scalar.activation(out=hid[:, mt, :], in_=ps[:, :], func=Act.Relu,
                                         bias=b1_sb[:, l * HT + mt: l * HT + mt + 1], scale=1.0)
            for mt in range(DT):
                ps = psmm.tile([P, CH], f32, tag="mm", name="ps2")
                for kt in range(HT):
                    nc.tensor.matmul(ps[:, :], w2_sb[:, kt, bass.ts(mt, P)], hid[:, kt, :],
                                     start=(kt == 0), stop=(kt == HT - 1))
                if out_f32_scalar_evac:
                    nc.scalar.activation(out=out_tile[:, mt, out_off: out_off + CH], in_=ps[:, :],
                                         func=Act.Identity,
                                         bias=b2_sb[:, l * DT + mt: l * DT + mt + 1], scale=1.0)
                else:
                    nc.vector.tensor_scalar(
                        out=out_tile[:, mt, out_off: out_off + CH], in0=ps[:, :],
                        scalar1=b2_sb[:, l * DT + mt: l * DT + mt + 1], scalar2=None,
                        op0=Alu.add)

        def conv_dw(m_t, l, dwd):
            """depthwise conv via diagonal-tap matmuls (TensorE) -> y bf16."""
            y = small.tile([P, DT, CH], bf16, tag="small", name="y")
            for d in range(DT):
                ps = psmm.tile([P, CH], f32, tag="mm", name="ps_dw")
                for j in range(K):
                    nc.tensor.matmul(ps[:, :], dwd[:, d, j, :], m_t[:, d, j: j + CH],
                                     start=(j == 0), stop=(j == K - 1))
                nc.vector.tensor_scalar(
                    out=y[:, d, :], in0=ps[:, :],
                    scalar1=dwb_sb[:, l * DT + d: l * DT + d + 1], scalar2=None,
                    op0=Alu.add)
            return y

        def conv_pw(y, l, pw_sb):
            """pointwise conv -> cv_bf (bf16 only; feeds the residual add)."""
            cv_bf = small.tile([P, DT, CH], bf16, tag="small", name="cv_bf")
            for mt in range(DT):
                ps = psmm.tile([P, CH], f32, tag="mm", name="ps3")
                for kt in range(DT):
                    nc.tensor.matmul(ps[:, :], pw_sb[:, kt, bass.ts(mt, P)], y[:, kt, :],
                                     start=(kt == 0), stop=(kt == DT - 1))
                nc.scalar.activation(out=cv_bf[:, mt, :], in_=ps[:, :], func=Act.Identity,
                                     bias=pwb_sb[:, l * DT + mt: l * DT + mt + 1], scale=1.0)
            return cv_bf

        def conv_pw_f32(y, l, pw_sb):
            """pointwise conv -> cv fp32 (layer 0: feeds LN directly)."""
            cv = big.tile([P, DT, CH], f32, tag="big", name="cv")
            for mt in range(DT):
                ps = psmm.tile([P, CH], f32, tag="mm", name="ps3")
                for kt in range(DT):
                    nc.tensor.matmul(ps[:, :], pw_sb[:, kt, bass.ts(mt, P)], y[:, kt, :],
                                     start=(kt == 0), stop=(kt == DT - 1))
                nc.scalar.activation(out=cv[:, mt, :], in_=ps[:, :], func=Act.Identity,
                                     bias=pwb_sb[:, l * DT + mt: l * DT + mt + 1], scale=1.0)
            return cv

        def gru_scan(rhs_bf, fw_sb, l, h_prev):
            """kh matmuls + gates + scan, per-d-tile pipelined. All gate/scan
            values fp32 (the h signal is a ~5e-4 variation on a 0.5 baseline;
            bf16 gate storage destroys it). Returns (hcol, h)."""
            h = big.tile([P, DT, CH], f32, tag="big", name="h")
            hcol = hcp.tile([P, DT, 1], f32, tag="hc", name="hcol")
            for d in range(DT):
                ps_k = psmm.tile([P, CH], f32, tag="mm", name="ps_k")
                for kt in range(DT):
                    nc.tensor.matmul(ps_k[:, :], fw_sb[:, kt, bass.ts(d, P)],
                                     rhs_bf[:, kt, :], start=(kt == 0), stop=(kt == DT - 1))
                ps_h = psmm.tile([P, CH], f32, tag="mm", name="ps_h")
                for kt in range(DT):
                    nc.tensor.matmul(ps_h[:, :], fw_sb[:, kt, bass.ts(DT + d, P)],
                                     rhs_bf[:, kt, :], start=(kt == 0), stop=(kt == DT - 1))
                kb_k = kb_sb[:, l * MT2 + d: l * MT2 + d + 1]
                kb_h = kb_sb[:, l * MT2 + DT + d: l * MT2 + DT + d + 1]
                z = zsp.tile([P, CH], f32, tag="zs", name="z")
                nc.scalar.activation(out=z[:, :], in_=ps_k[:, :], func=Act.Sigmoid,
                                     bias=kb_k, scale=1.0)
                cf = zsp.tile([P, CH], f32, tag="zs", name="cf")
                nc.scalar.activation(out=cf[:, :], in_=ps_k[:, :], func=Act.Sigmoid,
                                     bias=kb_k, scale=-1.0)
                s = zsp.tile([P, CH], f32, tag="zs", name="s")
                nc.scalar.activation(out=s[:, :], in_=ps_h[:, :], func=Act.Sigmoid,
                                     bias=kb_h, scale=1.0)
                nc.vector.scalar_tensor_tensor(
                    s[:, :], ps_h[:, :], 0.5, s[:, :], Alu.add, Alu.max)
                v = zsp.tile([P, CH], f32, tag="zs", name="v")
                nc.vector.tensor_mul(v[:, :], z[:, :], s[:, :])
                init = 0.5 if h_prev is None else h_prev[:, d, 0:1]
                nc.vector.tensor_tensor_scan(h[:, d, :], cf[:, :], v[:, :], init,
                                             Alu.mult, Alu.add)
            nc.vector.tensor_copy(out=hcol, in_=h[:, :, CH - 1: CH])
            return hcol, h

        def gru_res(h, base):
            """res = h + base, in place into h."""
            for d in range(DT):
                nc.vector.tensor_add(h[:, d, :], h[:, d, :], base[:, d, :])
            return h

        # wait-for-scale note on gates: scale=-1 sigmoid bias sign
        # cf = sigmoid(-(k + kb)) requires bias applied before negation:
        # activation computes func(scale*x + bias) so cf uses bias=-kb? see
        # host: we pass kb and use scale=-1 -> sigmoid(-k + kb) WRONG unless
        # kb == 0. Handled host-side: kbias is folded only when nonzero is
        # impossible (conv_pw_b zeros); assert there.

        # ---------- chunk programs ----------
        chunks = []
        wd0 = {}
        st0 = {"h": None}

        def mk_l0(c):
            def s0(_):
                if c == 0:
                    wd0["fw"] = load_w("fw", fwT, 0, [P, DT, E2], wpool2)
                    wd0["pw"] = load_w("pw", pwT, 0, [P, DT, D], wpool2)
                    wd0["dwd"] = load_dwd(0)
                if c == 1:
                    wd0["w1"] = load_w("w1", w1T, 0, [P, DT, H])
                    wd0["w2"] = load_w("w2", w2T, 0, [P, HT, D])
                x_in = small.tile([P, DT, CH + 3], bf16, tag="small", name="x_in")
                nc.sync.dma_start(out=x_in, in_=xT.ap().rearrange("(dt p) t -> p dt t", p=P)[:, :, c * CH: c * CH + CH + 3])
                return conv_dw(x_in, 0, wd0["dwd"])

            def s1(y):
                cv = conv_pw_f32(y, 0, wd0["pw"])
                S_ps, Q_ps = ln_stats(cv, shift=0.0)
                return cv, S_ps, Q_ps

            def s2(art):
                cv, S_ps, Q_ps = art
                n = ln_apply(cv, S_ps, Q_ps, 0, out_bf16=False, shift=0.0)
                n_bf = small.tile([P, DT, CH], bf16, tag="small", name="n_bf")
                for d in range(DT):
                    nc.scalar.activation(out=n_bf[:, d, :], in_=n[:, d, :], func=Act.Copy)
                return n, n_bf

            def s3(art):
                n, n_bf = art
                st0["h"], h = gru_scan(n_bf, wd0["fw"], 0, st0["h"])
                res = gru_res(h, n)
                nc.sync.dma_start(out=dram3(xs[0], c, CH), in_=res)

            return [s0, s1, s2, s3]

        wdm = [{} for _ in range(L - 1)]
        stm = [{"h": None, "m_prev": None} for _ in range(L - 1)]

        def mk_mid(i, c):
            wd, st = wdm[i], stm[i]
            src_d, dst_d = (xs[i], xs[i + 1]) if len(xs) == L else (xs[i % 2], xs[(i + 1) % 2])
            c_w12 = 1 if i == 0 else 2
            c_fwpw = 1 if i == 0 else 3

            def s0(_):
                if i > 0 and c == c_w12:
                    wd["w1"] = load_w("w1", w1T, i, [P, DT, H])
                    wd["w2"] = load_w("w2", w2T, i, [P, HT, D])
                if c == c_fwpw:
                    wd["fw"] = load_w("fw", fwT, i + 1, [P, DT, E2], wpool2)
                    wd["pw"] = load_w("pw", pwT, i + 1, [P, DT, D], wpool2)
                    wd["dwd"] = load_dwd(i + 1)
                x_in = big.tile([P, DT, CH], f32, tag="big", name="x_in")
                nc.sync.dma_start(out=x_in, in_=dram3(src_d, c, CH))
                S_ps, Q_ps = ln_stats(x_in, shift=0.5)
                return x_in, S_ps, Q_ps

            def s1(art):
                x_in, S_ps, Q_ps = art
                return ln_apply(x_in, S_ps, Q_ps, 1 + i, out_bf16=True, shift=0.5)

            def s2(a):
                if i == 0:
                    w1_sb, w2_sb = wd0["w1"], wd0["w2"]
                else:
                    w1_sb, w2_sb = wd["w1"], wd["w2"]
                m = mp.tile([P, DT, CH + 3], bf16, tag="m_bf", name="m")
                mlp_chunk(a, i, w1_sb, w2_sb, m, 3, out_f32_scalar_evac=False)
                if c == 0:
                    nc.vector.memset(m[:, :, 0:3], 0.0)
                else:
                    nc.vector.tensor_copy(out=m[:, :, 0:3], in_=st["m_prev"][:, :, CH: CH + 3])
                st["m_prev"] = m
                return m

            def s3(m):
                y = conv_dw(m, i + 1, wd["dwd"])
                st["h"], h = gru_scan(y, wd["fw"], i + 1, st["h"])
                return y, h

            def s4(art):
                y, h = art
                cv_bf = conv_pw(y, i + 1, wd["pw"])
                res = gru_res(h, cv_bf)
                nc.sync.dma_start(out=dram3(dst_d, c, CH), in_=res)

            return [s0, s1, s2, s3, s4]

        wdt = {}
        src_t = xs[L - 1] if len(xs) == L else xs[(L - 1) % 2]

        def mk_tail(c):
            def s0(_):
                if c == 2:
                    wdt["w1"] = load_w("w1", w1T, L - 1, [P, DT, H])
                    wdt["w2"] = load_w("w2", w2T, L - 1, [P, HT, D])
                x_in = big.tile([P, DT, CH], f32, tag="big", name="x_in")
                nc.sync.dma_start(out=x_in, in_=dram3(src_t, c, CH))
                S_ps, Q_ps = ln_stats(x_in, shift=0.5)
                return x_in, S_ps, Q_ps

            def s1(art):
                x_in, S_ps, Q_ps = art
                return ln_apply(x_in, S_ps, Q_ps, L, out_bf16=True, shift=0.5,
                                dump=(dbg if (debug_outs and c == 0) else None))

            def s2(a):
                o = big.tile([P, DT, CH], f32, tag="big", name="o")
                mlp_chunk(a, L - 1, wdt["w1"], wdt["w2"], o, 0, out_f32_scalar_evac=True)
                nc.sync.dma_start(out=dram3(out_t, c, CH), in_=o)

            return [s0, s1, s2]

        # ---------- emission: interleave L0 into M0, then serial ----------
        # L0c0..L0c3 M0c0 L0c4 M0c1 L0c5 ... L0c7 M0c4..M0c7 M1c0..7 ...
        # M0's chunk c load must land strictly after L0's chunk-c store
        # (pos(M0c) >= pos(L0c)+4 keeps the DRAM RAW ordering).
        INTERLEAVE = True
        if INTERLEAVE:
            for c in range(4):
                chunks.append(mk_l0(c))
            for c in range(NCH - 4):
                chunks.append(mk_mid(0, c))
                chunks.append(mk_l0(c + 4))
            for c in range(NCH - 4, NCH):
                chunks.append(mk_mid(0, c))
        else:
            for c in range(NCH):
                chunks.append(mk_l0(c))
            for c in range(NCH):
                chunks.append(mk_mid(0, c))
        for i in range(1, L - 1):
            for c in range(NCH):
                chunks.append(mk_mid(i, c))
        for c in range(NCH):
            chunks.append(mk_tail(c))

        NST = 5
        arts = [None] * len(chunks)
        for g in range(len(chunks) + NST - 1):
            for k in range(NST):
                idx = g - k
                if 0 <= idx < len(chunks) and k < len(chunks[idx]):
                    arts[idx] = chunks[idx][k](arts[idx])

    return nc


_CACHE = {}


def get_compiled_nc(T=4096, CH=512, has_lnb=False, **kw):
    key = (T, CH, has_lnb, tuple(sorted(kw.items())))
    if key not in _CACHE:
        nc = build_nc(T, CH, has_lnb, **kw)
        nc.compile()
        _CACHE[key] = nc
    return _CACHE[key]


def _make_dw_diag(dw_w):
    """dw_w (L, K, D) -> diagonal-tap lhsT tiles (L, 128, DT*K*128) bf16.
    arr[l, k, dt, j, c] = (k==c) * dw_w[l, j, dt*128+c]."""
    DT = D // P
    arr = np.zeros((L, P, DT, K, P), np.float32)
    idx = np.arange(P)
    for l in range(L):
        for dt in range(DT):
            for j in range(K):
                arr[l, idx, dt, j, idx] = dw_w[l, j, dt * P + idx]
    return arr.reshape(L, P, DT * K * P).astype(BF)


def make_host_inputs(inputs, T=4096):
    f = np.float32
    fw = np.asarray(inputs["f_w"], np.float64)          # (L, 2D, D)
    pw = np.asarray(inputs["conv_pw_w"], np.float64)    # (L, D, D)
    pwb = np.asarray(inputs["conv_pw_b"], np.float64)   # (L, D)
    # Fused FW' for layers 1..L-1: kh = f_w @ (pw @ y + pwb) = (f_w@pw) @ y + f_w@pwb
    fw_eff = fw.copy()
    kb = np.zeros((L, 2 * D), np.float64)
    for l in range(1, L):
        fw_eff[l] = fw[l] @ pw[l]
        kb[l] = fw[l] @ pwb[l]
    assert np.abs(kb).max() == 0.0, "nonzero conv_pw_b needs kb sign handling in gates"
    w = {
        "fwT": np.ascontiguousarray(np.transpose(fw_eff, (0, 2, 1))).astype(BF),
        "pwT": np.ascontiguousarray(np.transpose(np.asarray(inputs["conv_pw_w"], f), (0, 2, 1))).astype(BF),
        "w1T": np.ascontiguousarray(np.transpose(np.asarray(inputs["mlp_w1"], f), (0, 2, 1))).astype(BF),
        "w2T": np.ascontiguousarray(np.transpose(np.asarray(inputs["mlp_w2"], f), (0, 2, 1))).astype(BF),
        "dwDg": _make_dw_diag(np.asarray(inputs["conv_dw_w"], f)),
        "dwb": np.asarray(inputs["conv_dw_b"], f),
        "pwb": np.asarray(inputs["conv_pw_b"], f),
        "kbv": kb.astype(f),
        "b1v": np.asarray(inputs["mlp_b1"], f),
        "b2v": np.asarray(inputs["mlp_b2"], f),
        "lng": np.concatenate([np.asarray(inputs["ln1_g"], f)[None], np.asarray(inputs["ln2_g"], f)], 0),
        "lnb": np.concatenate([np.asarray(inputs["ln1_b"], f)[None], np.asarray(inputs["ln2_b"], f)], 0),
    }
    x = np.asarray(inputs["x"], f)
    nb = x.shape[0]
    in_maps = []
    for b in range(nb):
        xTp = np.zeros((D, T + 3), BF)
        xTp[:, 3:] = x[b, :T].T.astype(BF)
        in_maps.append({"xT": xTp, **w})
    has_lnb = bool(np.any(w["lnb"] != 0.0))
    return in_maps, has_lnb


def kernel(**inputs):
    from concourse.bass_utils import run_bass_kernel_spmd

    T = int(np.asarray(inputs["x"]).shape[1])
    in_maps, has_lnb = make_host_inputs(inputs, T)
    nc = get_compiled_nc(T=T, has_lnb=has_lnb)
    res = run_bass_kernel_spmd(nc, in_maps, core_ids=list(range(len(in_maps))))
    out = np.stack([r["out"].T for r in res.results])
    return np.ascontiguousarray(out.astype(np.float32))


# revision 26
# speedup vs baseline: 1.0878x; 1.0878x over previous
"""Trainium2 Bass kernel for nn_BlockV2 (conv -> LN -> minGRU -> MLP x4).

Strategy: data-parallel over batch (B=8 -> 8 cores). Per core, activations
are kept in [D_partitions, T_free] layout and streamed through each layer in
chunks of 512 tokens; inter-layer activations ping-pong through DRAM.
The minGRU recurrence h_t = c_t*h_{t-1} + v_t runs on the VectorE
tensor_tensor_scan instruction (fp32 state), chained across chunks.

v2 changes vs baseline:
- LN statistics matmuls run in bf16 (the fp32 ones were LOW_HIGH two-pass,
  ~4x the cost); the bf16 stat input copies are made on the idle GpSimd
  engine, and the per-token mean/rstd broadcasts use gpsimd
  partition_broadcast instead of TensorE ones-matmuls.
- f_w@conv_pw_w is fused host-side (FW'), so the mid-layer GRU kh matmul
  consumes the depthwise-conv output y directly - the pointwise conv output
  cv is only needed (in bf16) for the residual add.
- conv_dw runs fully in bf16 on VectorE (2x rate); the MLP output tile m is
  stored bf16 (it only feeds conv_dw).
- layer-0 chunks (TensorE-light, VectorE-heavy) are interleaved into the
  first mid layer's chunk stream so TensorE never starves during the ramp.
- PSUM evacuations are split between ScalarE and VectorE.
"""
import sys

sys.path.insert(0, "/opt/trn_rl_repo")

from contextlib import ExitStack

import numpy as np
import ml_dtypes

import concourse.bass as bass
import concourse.tile as tile
from concourse import bacc, mybir

f32 = mybir.dt.float32
bf16 = mybir.dt.bfloat16
Alu = mybir.AluOpType
Act = mybir.ActivationFunctionType
BF = ml_dtypes.bfloat16

B, D, L, K, H = 8, 512, 4, 4, 2048
N_CORES = 8
LN_EPS = 1e-5
P = 128


def build_nc(T=4096, CH=512, has_lnb=False, debug_outs=False, use_gpsimd=False):
    NCH = T // CH
    DT = D // P      # 4 d-tiles
    HT = H // P      # 16 h-tiles
    E2 = 2 * D
    MT2 = E2 // P    # 8 m-tiles of the kh matmul

    nc = bacc.Bacc("TRN2", target_bir_lowering=False, debug=False)

    xT = nc.dram_tensor("xT", [D, T + 3], bf16, kind="ExternalInput")
    fwT = nc.dram_tensor("fwT", [L, D, E2], bf16, kind="ExternalInput")
    pwT = nc.dram_tensor("pwT", [L, D, D], bf16, kind="ExternalInput")
    w1T = nc.dram_tensor("w1T", [L, D, H], bf16, kind="ExternalInput")
    w2T = nc.dram_tensor("w2T", [L, H, D], bf16, kind="ExternalInput")
    dwK = nc.dram_tensor("dwK", [L, D, K], f32, kind="ExternalInput")
    dwb = nc.dram_tensor("dwb", [L, D], f32, kind="ExternalInput")
    pwb = nc.dram_tensor("pwb", [L, D], f32, kind="ExternalInput")
    kbv = nc.dram_tensor("kbv", [L, E2], f32, kind="ExternalInput")
    b1v = nc.dram_tensor("b1v", [L, H], f32, kind="ExternalInput")
    b2v = nc.dram_tensor("b2v", [L, D], f32, kind="ExternalInput")
    lng = nc.dram_tensor("lng", [L + 1, D], f32, kind="ExternalInput")
    lnb = nc.dram_tensor("lnb", [L + 1, D], f32, kind="ExternalInput")
    out_t = nc.dram_tensor("out", [D, T], f32, kind="ExternalOutput")
    if debug_outs:
        xs = [nc.dram_tensor(f"xs{i}", [D, T], f32, kind="ExternalOutput")
              for i in range(L)]
        dbg = {
            "mu": nc.dram_tensor("dbg_mu", [1, 512], f32, kind="ExternalOutput"),
            "var": nc.dram_tensor("dbg_var", [1, 512], f32, kind="ExternalOutput"),
            "rstd": nc.dram_tensor("dbg_rstd", [1, 512], f32, kind="ExternalOutput"),
            "a": nc.dram_tensor("dbg_a", [D, 512], f32, kind="ExternalOutput"),
        }
    else:
        dbg = None
        xs = [nc.dram_tensor(f"xs{i}", [D, T], f32) for i in range(2)]

    def dram3(tensor, c, width):
        return tensor.ap().rearrange("(dt p) t -> p dt t", p=P)[:, :, c * CH: c * CH + width]

    with tile.TileContext(nc) as tc, ExitStack() as ctx:
        sing = ctx.enter_context(tc.tile_pool(name="sing", bufs=1))
        wpool = ctx.enter_context(tc.tile_pool(name="w", bufs=1))
        wpool2 = ctx.enter_context(tc.tile_pool(name="w2", bufs=2))
        big = ctx.enter_context(tc.tile_pool(name="big", bufs=7))
        small = ctx.enter_context(tc.tile_pool(name="small", bufs=8))
        mp = ctx.enter_context(tc.tile_pool(name="mp", bufs=2))
        hidp = ctx.enter_context(tc.tile_pool(name="hid", bufs=1))
        statp = ctx.enter_context(tc.tile_pool(name="stat", bufs=5))
        hcp = ctx.enter_context(tc.tile_pool(name="hc", bufs=2))
        zsp = ctx.enter_context(tc.tile_pool(name="zs", bufs=8))
        bcp = ctx.enter_context(tc.tile_pool(name="bc", bufs=2))
        psmm = ctx.enter_context(tc.tile_pool(name="psmm", bufs=6, space="PSUM"))
        psst = ctx.enter_context(tc.tile_pool(name="psst", bufs=2, space="PSUM"))


        ones_col = sing.tile([P, 1], bf16)
        nc.vector.memset(ones_col, 1.0)
        ones_row = sing.tile([1, P], bf16)
        nc.vector.memset(ones_row, 1.0)
        eps1 = sing.tile([1, 1], f32)
        nc.vector.memset(eps1, LN_EPS)
        zero1 = sing.tile([1, 1], f32)
        nc.vector.memset(zero1, 0.0)
        nhalf_col = sing.tile([P, 1], f32)
        nc.vector.memset(nhalf_col, -0.5)
        zero_col = sing.tile([P, 1], f32)
        nc.vector.memset(zero_col, 0.0)
        dw_sb = sing.tile([P, L * DT, K], f32)
        nc.sync.dma_start(out=dw_sb, in_=dwK.ap().rearrange("l (dt p) k -> p (l dt) k", p=P))
        dwb_sb = sing.tile([P, L * DT], f32)
        nc.sync.dma_start(out=dwb_sb, in_=dwb.ap().rearrange("l (dt p) -> p (l dt)", p=P))
        pwb_sb = sing.tile([P, L * DT], f32)
        nc.sync.dma_start(out=pwb_sb, in_=pwb.ap().rearrange("l (dt p) -> p (l dt)", p=P))
        kb_sb = sing.tile([P, L * MT2], f32)
        nc.sync.dma_start(out=kb_sb, in_=kbv.ap().rearrange("l (mt p) -> p (l mt)", p=P))
        b1_sb = sing.tile([P, L * HT], f32)
        nc.sync.dma_start(out=b1_sb, in_=b1v.ap().rearrange("l (ht p) -> p (l ht)", p=P))
        b2_sb = sing.tile([P, L * DT], f32)
        nc.sync.dma_start(out=b2_sb, in_=b2v.ap().rearrange("l (dt p) -> p (l dt)", p=P))
        lng_sb = sing.tile([P, (L + 1) * DT], f32)
        nc.sync.dma_start(out=lng_sb, in_=lng.ap().rearrange("l (dt p) -> p (l dt)", p=P))
        lnb_sb = sing.tile([P, (L + 1) * DT], f32)
        nc.sync.dma_start(out=lnb_sb, in_=lnb.ap().rearrange("l (dt p) -> p (l dt)", p=P))

        def load_w(kind, dram, l, shape, pool=wpool):
            t = pool.tile(shape, bf16, tag=kind, name=f"{kind}{l}")
            nc.sync.dma_start(out=t, in_=dram.ap()[l].rearrange("(kt p) e -> p kt e", p=P))
            return t


        # ---------- LN pieces ----------
        # The residual-stream LN inputs sit on a ~0.5 DC baseline with tiny
        # per-token variance (down to ~1e-5 at the last layer), so the stats
        # are computed on x' = x - shift (exact in fp32, then bf16): the
        # E[x'^2] - mu'^2 cancellation is benign once the DC is removed.
        def ln_stats(x_tile, shift):
            """shifted bf16 copy + xsq + S'/Q' bf16 matmuls. Returns psum tiles."""
            x_bf = small.tile([P, DT, CH], bf16, tag="small", name="x_bf")
            for d in range(DT):
                if d % 2 == 0:
                    sb = nhalf_col if shift == 0.5 else zero_col
                    nc.scalar.activation(out=x_bf[:, d, :], in_=x_tile[:, d, :],
                                         func=Act.Identity, bias=sb[:, :], scale=1.0)
                else:
                    nc.vector.tensor_scalar(
                        out=x_bf[:, d, :], in0=x_tile[:, d, :],
                        scalar1=-shift, scalar2=None, op0=Alu.add)
            xsq = small.tile([P, DT, CH], bf16, tag="small", name="xsq")
            for d in range(DT):
                nc.vector.tensor_mul(xsq[:, d, :], x_bf[:, d, :], x_bf[:, d, :])
            S_ps = psst.tile([1, CH], f32, tag="ps_stat", name="S_ps")
            for kt in range(DT):
                nc.tensor.matmul(S_ps[:, :], ones_col[:, :], x_bf[:, kt, :],
                                 start=(kt == 0), stop=(kt == DT - 1))
            Q_ps = psst.tile([1, CH], f32, tag="ps_stat", name="Q_ps")
            for kt in range(DT):
                nc.tensor.matmul(Q_ps[:, :], ones_col[:, :], xsq[:, kt, :],
                                 start=(kt == 0), stop=(kt == DT - 1))
            return S_ps, Q_ps

        def ln_apply(x_tile, S_ps, Q_ps, slot, out_bf16, shift, dump=None):
            """stat algebra + gpsimd broadcasts + center-in-place + scale.
            mu' = mean(x - shift); center computes (x - shift) - mu'."""
            mu = statp.tile([1, CH], f32, tag="stat", name="mu")
            nc.scalar.activation(out=mu[:, :], in_=S_ps[:, :], func=Act.Identity,
                                 bias=0.0, scale=1.0 / D)
            musq = statp.tile([1, CH], f32, tag="stat", name="musq")
            nc.scalar.activation(out=musq[:, :], in_=mu[:, :], func=Act.Square,
                                 bias=0.0, scale=1.0)
            var = statp.tile([1, CH], f32, tag="stat", name="var")
            nc.vector.scalar_tensor_tensor(
                var[:, :], Q_ps[:, :], 1.0 / D, musq[:, :], Alu.mult, Alu.subtract)
            lnv = statp.tile([1, CH], f32, tag="stat", name="lnv")
            nc.scalar.activation(out=lnv[:, :], in_=var[:, :], func=Act.Ln,
                                 bias=eps1[:, :], scale=1.0)
            rstd = statp.tile([1, CH], f32, tag="stat", name="rstd")
            nc.scalar.activation(out=rstd[:, :], in_=lnv[:, :], func=Act.Exp, scale=-0.5)
            bc_mu = bcp.tile([P, CH], f32, tag="bc", name="bc_mu")
            nc.gpsimd.partition_broadcast(bc_mu[:, :], mu[:, :])
            bc_r = bcp.tile([P, CH], f32, tag="bc", name="bc_r")
            nc.gpsimd.partition_broadcast(bc_r[:, :], rstd[:, :])
            for d in range(DT):
                nc.vector.scalar_tensor_tensor(
                    x_tile[:, d, :], x_tile[:, d, :], -shift, bc_mu[:, :],
                    Alu.add, Alu.subtract)
            if out_bf16:
                a_t = small.tile([P, DT, CH], bf16, tag="small", name="a_t")
            else:
                a_t = big.tile([P, DT, CH], f32, tag="big", name="a_t")
            for d in range(DT):
                nc.vector.scalar_tensor_tensor(
                    a_t[:, d, :], x_tile[:, d, :], lng_sb[:, slot * DT + d: slot * DT + d + 1],
                    bc_r[:, :], Alu.mult, Alu.mult)
            if dump is not None:
                nc.sync.dma_start(out=dump["mu"].ap(), in_=mu)
                nc.sync.dma_start(out=dump["var"].ap(), in_=var)
                nc.sync.dma_start(out=dump["rstd"].ap(), in_=rstd)
                a_f32 = big.tile([P, DT, CH], f32, tag="big", name="a_f32")
                for d in range(DT):
                    nc.vector.tensor_copy(out=a_f32[:, d, :], in_=a_t[:, d, :])
                nc.sync.dma_start(out=dump["a"].ap().rearrange("(dt p) t -> p dt t", p=P), in_=a_f32)
            if has_lnb:
                for d in range(DT):
                    nc.vector.tensor_scalar(
                        out=a_t[:, d, :], in0=a_t[:, d, :],
                        scalar1=lnb_sb[:, slot * DT + d: slot * DT + d + 1], scalar2=None,
                        op0=Alu.add)
            return a_t

        def mlp_chunk(a_t, l, w1_sb, w2_sb, out_tile, out_off, out_f32_scalar_evac):
            hid = hidp.tile([P, HT, CH], bf16, tag="hid", name="hid")
            for mt in range(HT):
                ps = psmm.tile([P, CH], f32, tag="mm", name="ps1")
                for kt in range(DT):
                    nc.tensor.matmul(ps[:, :], w1_sb[:, kt, bass.ts(mt, P)], a_t[:, kt, :],
                                     start=(kt == 0), stop=(kt == DT - 1))
                RELU_SPLIT = False
                if RELU_SPLIT and mt % 2 == 1:
                    nc.vector.tensor_scalar(
                        out=hid[:, mt, :], in0=ps[:, :],
                        scalar1=b1_sb[:, l * HT + mt: l * HT + mt + 1], scalar2=0.0,
                        op0=Alu.add, op1=Alu.max)
                else:
                    nc.scalar.activation(out=hid[:, mt, :], in_=ps[:, :], func=Act.Relu,
                                         bias=b1_sb[:, l * HT + mt: l * HT + mt + 1], scale=1.0)
            for mt in range(DT):
                ps = psmm.tile([P, CH], f32, tag="mm", name="ps2")
                for kt in range(HT):
                    nc.tensor.matmul(ps[:, :], w2_sb[:, kt, bass.ts(mt, P)], hid[:, kt, :],
                                     start=(kt == 0), stop=(kt == HT - 1))
                if out_f32_scalar_evac:
                    nc.scalar.activation(out=out_tile[:, mt, out_off: out_off + CH], in_=ps[:, :],
                                         func=Act.Identity,
                                         bias=b2_sb[:, l * DT + mt: l * DT + mt + 1], scale=1.0)
                else:
                    nc.vector.tensor_scalar(
                        out=out_tile[:, mt, out_off: out_off + CH], in0=ps[:, :],
                        scalar1=b2_sb[:, l * DT + mt: l * DT + mt + 1], scalar2=None,
                        op0=Alu.add)

        def conv_dw(m_t, l):
            """depthwise conv over bf16 input tile [P, DT, CH+3] -> y bf16."""
            acc = small.tile([P, DT, CH], bf16, tag="small", name="acc")
            y = small.tile([P, DT, CH], bf16, tag="small", name="y")
            for d in range(DT):
                nc.vector.tensor_scalar(
                    out=acc[:, d, :], in0=m_t[:, d, 0: CH],
                    scalar1=dw_sb[:, l * DT + d, 0:1], scalar2=dwb_sb[:, l * DT + d: l * DT + d + 1],
                    op0=Alu.mult, op1=Alu.add)
                for j in range(1, K - 1):
                    nc.vector.scalar_tensor_tensor(
                        acc[:, d, :], m_t[:, d, j: j + CH], dw_sb[:, l * DT + d, j: j + 1],
                        acc[:, d, :], Alu.mult, Alu.add)
                nc.vector.scalar_tensor_tensor(
                    y[:, d, :], m_t[:, d, K - 1: K - 1 + CH], dw_sb[:, l * DT + d, K - 1: K],
                    acc[:, d, :], Alu.mult, Alu.add)
            return y

        def conv_pw(y, l, pw_sb):
            """pointwise conv -> cv_bf (bf16 only; feeds the residual add)."""
            cv_bf = small.tile([P, DT, CH], bf16, tag="small", name="cv_bf")
            for mt in range(DT):
                ps = psmm.tile([P, CH], f32, tag="mm", name="ps3")
                for kt in range(DT):
                    nc.tensor.matmul(ps[:, :], pw_sb[:, kt, bass.ts(mt, P)], y[:, kt, :],
                                     start=(kt == 0), stop=(kt == DT - 1))
                nc.scalar.activation(out=cv_bf[:, mt, :], in_=ps[:, :], func=Act.Identity,
                                     bias=pwb_sb[:, l * DT + mt: l * DT + mt + 1], scale=1.0)
            return cv_bf

        def conv_pw_f32(y, l, pw_sb):
            """pointwise conv -> cv fp32 (layer 0: feeds LN directly)."""
            cv = big.tile([P, DT, CH], f32, tag="big", name="cv")
            for mt in range(DT):
                ps = psmm.tile([P, CH], f32, tag="mm", name="ps3")
                for kt in range(DT):
                    nc.tensor.matmul(ps[:, :], pw_sb[:, kt, bass.ts(mt, P)], y[:, kt, :],
                                     start=(kt == 0), stop=(kt == DT - 1))
                nc.scalar.activation(out=cv[:, mt, :], in_=ps[:, :], func=Act.Identity,
                                     bias=pwb_sb[:, l * DT + mt: l * DT + mt + 1], scale=1.0)
            return cv

        def gru_scan(rhs_bf, fw_sb, l, h_prev):
            """kh matmuls + gates + scan, per-d-tile pipelined. All gate/scan
            values fp32 (the h signal is a ~5e-4 variation on a 0.5 baseline;
            bf16 gate storage destroys it). Returns (hcol, h)."""
            h = big.tile([P, DT, CH], f32, tag="big", name="h")
            hcol = hcp.tile([P, DT, 1], f32, tag="hc", name="hcol")
            for d in range(DT):
                ps_k = psmm.tile([P, CH], f32, tag="mm", name="ps_k")
                for kt in range(DT):
                    nc.tensor.matmul(ps_k[:, :], fw_sb[:, kt, bass.ts(d, P)],
                                     rhs_bf[:, kt, :], start=(kt == 0), stop=(kt == DT - 1))
                ps_h = psmm.tile([P, CH], f32, tag="mm", name="ps_h")
                for kt in range(DT):
                    nc.tensor.matmul(ps_h[:, :], fw_sb[:, kt, bass.ts(DT + d, P)],
                                     rhs_bf[:, kt, :], start=(kt == 0), stop=(kt == DT - 1))
                kb_k = kb_sb[:, l * MT2 + d: l * MT2 + d + 1]
                kb_h = kb_sb[:, l * MT2 + DT + d: l * MT2 + DT + d + 1]
                z = zsp.tile([P, CH], f32, tag="zs", name="z")
                nc.scalar.activation(out=z[:, :], in_=ps_k[:, :], func=Act.Sigmoid,
                                     bias=kb_k, scale=1.0)
                cf = zsp.tile([P, CH], f32, tag="zs", name="cf")
                nc.scalar.activation(out=cf[:, :], in_=ps_k[:, :], func=Act.Sigmoid,
                                     bias=kb_k, scale=-1.0)
                s = zsp.tile([P, CH], f32, tag="zs", name="s")
                nc.scalar.activation(out=s[:, :], in_=ps_h[:, :], func=Act.Sigmoid,
                                     bias=kb_h, scale=1.0)
                nc.vector.scalar_tensor_tensor(
                    s[:, :], ps_h[:, :], 0.5, s[:, :], Alu.add, Alu.max)
                v = zsp.tile([P, CH], f32, tag="zs", name="v")
                nc.vector.tensor_mul(v[:, :], z[:, :], s[:, :])
                init = 0.5 if h_prev is None else h_prev[:, d, 0:1]
                nc.vector.tensor_tensor_scan(h[:, d, :], cf[:, :], v[:, :], init,
                                             Alu.mult, Alu.add)
            nc.vector.tensor_copy(out=hcol, in_=h[:, :, CH - 1: CH])
            return hcol, h

        def gru_res(h, base):
            """res = h + base, in place into h."""
            for d in range(DT):
                nc.vector.tensor_add(h[:, d, :], h[:, d, :], base[:, d, :])
            return h

        # wait-for-scale note on gates: scale=-1 sigmoid bias sign
        # cf = sigmoid(-(k + kb)) requires bias applied before negation:
        # activation computes func(scale*x + bias) so cf uses bias=-kb? see
        # host: we pass kb and use scale=-1 -> sigmoid(-k + kb) WRONG unless
        # kb == 0. Handled host-side: kbias is folded only when nonzero is
        # impossible (conv_pw_b zeros); assert there.

        # ---------- chunk programs ----------
        chunks = []
        wd0 = {}
        st0 = {"h": None}

        def mk_l0(c):
            def s0(_):
                if c == 0:
                    wd0["fw"] = load_w("fw", fwT, 0, [P, DT, E2], wpool2)
                    wd0["pw"] = load_w("pw", pwT, 0, [P, DT, D], wpool2)
                if c == 1:
                    wd0["w1"] = load_w("w1", w1T, 0, [P, DT, H])
                    wd0["w2"] = load_w("w2", w2T, 0, [P, HT, D])
                x_in = small.tile([P, DT, CH + 3], bf16, tag="small", name="x_in")
                nc.sync.dma_start(out=x_in, in_=xT.ap().rearrange("(dt p) t -> p dt t", p=P)[:, :, c * CH: c * CH + CH + 3])
                return conv_dw(x_in, 0)

            def s1(y):
                cv = conv_pw_f32(y, 0, wd0["pw"])
                S_ps, Q_ps = ln_stats(cv, shift=0.0)
                return cv, S_ps, Q_ps

            def s2(art):
                cv, S_ps, Q_ps = art
                n = ln_apply(cv, S_ps, Q_ps, 0, out_bf16=False, shift=0.0)
                n_bf = small.tile([P, DT, CH], bf16, tag="small", name="n_bf")
                for d in range(DT):
                    nc.gpsimd.tensor_copy(out=n_bf[:, d, :], in_=n[:, d, :])
                return n, n_bf

            def s3(art):
                n, n_bf = art
                st0["h"], h = gru_scan(n_bf, wd0["fw"], 0, st0["h"])
                res = gru_res(h, n)
                nc.sync.dma_start(out=dram3(xs[0], c, CH), in_=res)

            return [s0, s1, s2, s3]

        wdm = [{} for _ in range(L - 1)]
        stm = [{"h": None, "m_prev": None} for _ in range(L - 1)]

        def mk_mid(i, c):
            wd, st = wdm[i], stm[i]
            src_d, dst_d = (xs[i], xs[i + 1]) if len(xs) == L else (xs[i % 2], xs[(i + 1) % 2])
            c_w12 = 1 if i == 0 else 2
            c_fwpw = 1 if i == 0 else 3

            def s0(_):
                if i > 0 and c == c_w12:
                    wd["w1"] = load_w("w1", w1T, i, [P, DT, H])
                    wd["w2"] = load_w("w2", w2T, i, [P, HT, D])
                if c == c_fwpw:
                    wd["fw"] = load_w("fw", fwT, i + 1, [P, DT, E2], wpool2)
                    wd["pw"] = load_w("pw", pwT, i + 1, [P, DT, D], wpool2)
                x_in = big.tile([P, DT, CH], f32, tag="big", name="x_in")
                nc.sync.dma_start(out=x_in, in_=dram3(src_d, c, CH))
                S_ps, Q_ps = ln_stats(x_in, shift=0.5)
                return x_in, S_ps, Q_ps

            def s1(art):
                x_in, S_ps, Q_ps = art
                return ln_apply(x_in, S_ps, Q_ps, 1 + i, out_bf16=True, shift=0.5)

            def s2(a):
                if i == 0:
                    w1_sb, w2_sb = wd0["w1"], wd0["w2"]
                else:
                    w1_sb, w2_sb = wd["w1"], wd["w2"]
                m = mp.tile([P, DT, CH + 3], bf16, tag="m_bf", name="m")
                mlp_chunk(a, i, w1_sb, w2_sb, m, 3, out_f32_scalar_evac=False)
                if c == 0:
                    nc.vector.memset(m[:, :, 0:3], 0.0)
                else:
                    nc.vector.tensor_copy(out=m[:, :, 0:3], in_=st["m_prev"][:, :, CH: CH + 3])
                st["m_prev"] = m
                return m

            def s3(m):
                y = conv_dw(m, i + 1)
                st["h"], h = gru_scan(y, wd["fw"], i + 1, st["h"])
                return y, h

            def s4(art):
                y, h = art
                cv_bf = conv_pw(y, i + 1, wd["pw"])
                res = gru_res(h, cv_bf)
                nc.sync.dma_start(out=dram3(dst_d, c, CH), in_=res)

            return [s0, s1, s2, s3, s4]

        wdt = {}
        src_t = xs[L - 1] if len(xs) == L else xs[(L - 1) % 2]

        def mk_tail(c):
            def s0(_):
                if c == 2:
                    wdt["w1"] = load_w("w1", w1T, L - 1, [P, DT, H])
                    wdt["w2"] = load_w("w2", w2T, L - 1, [P, HT, D])
                x_in = big.tile([P, DT, CH], f32, tag="big", name="x_in")
                nc.sync.dma_start(out=x_in, in_=dram3(src_t, c, CH))
                S_ps, Q_ps = ln_stats(x_in, shift=0.5)
                return x_in, S_ps, Q_ps

            def s1(art):
                x_in, S_ps, Q_ps = art
                return ln_apply(x_in, S_ps, Q_ps, L, out_bf16=True, shift=0.5,
                                dump=(dbg if (debug_outs and c == 0) else None))

            def s2(a):
                o = big.tile([P, DT, CH], f32, tag="big", name="o")
                mlp_chunk(a, L - 1, wdt["w1"], wdt["w2"], o, 0, out_f32_scalar_evac=True)
                nc.sync.dma_start(out=dram3(out_t, c, CH), in_=o)

            return [s0, s1, s2]

        # ---------- emission: interleave L0 into M0, then serial ----------
        # L0c0..L0c3 M0c0 L0c4 M0c1 L0c5 ... L0c7 M0c4..M0c7 M1c0..7 ...
        # M0's chunk c load must land strictly after L0's chunk-c store
        # (pos(M0c) >= pos(L0c)+4 keeps the DRAM RAW ordering).
        INTERLEAVE = True
        if INTERLEAVE:
            for c in range(4):
                chunks.append(mk_l0(c))
            for c in range(NCH - 4):
                chunks.append(mk_mid(0, c))
                chunks.append(mk_l0(c + 4))
            for c in range(NCH - 4, NCH):
                chunks.append(mk_mid(0, c))
        else:
            for c in range(NCH):
                chunks.append(mk_l0(c))
            for c in range(NCH):
                chunks.append(mk_mid(0, c))
        for i in range(1, L - 1):
            for c in range(NCH):
                chunks.append(mk_mid(i, c))
        for c in range(NCH):
            chunks.append(mk_tail(c))

        NST = 5
        arts = [None] * len(chunks)
        for g in range(len(chunks) + NST - 1):
            for k in range(NST):
                idx = g - k
                if 0 <= idx < len(chunks) and k < len(chunks[idx]):
                    arts[idx] = chunks[idx][k](arts[idx])

    return nc


_CACHE = {}


def get_compiled_nc(T=4096, CH=512, has_lnb=False, **kw):
    key = (T, CH, has_lnb, tuple(sorted(kw.items())))
    if key not in _CACHE:
        nc = build_nc(T, CH, has_lnb, **kw)
        nc.compile()
        _CACHE[key] = nc
    return _CACHE[key]


def _make_dw_diag(dw_w):
    """dw_w (L, K, D) -> diagonal-tap lhsT tiles (L, 128, DT*K*128) bf16.
    arr[l, k, dt, j, c] = (k==c) * dw_w[l, j, dt*128+c]."""
    DT = D // P
    arr = np.zeros((L, P, DT, K, P), np.float32)
    idx = np.arange(P)
    for l in range(L):
        for dt in range(DT):
            for j in range(K):
                arr[l, idx, dt, j, idx] = dw_w[l, j, dt * P + idx]
    return arr.reshape(L, P, DT * K * P).astype(BF)


def make_host_inputs(inputs, T=4096):
    f = np.float32
    fw = np.asarray(inputs["f_w"], np.float64)          # (L, 2D, D)
    pw = np.asarray(inputs["conv_pw_w"], np.float64)    # (L, D, D)
    pwb = np.asarray(inputs["conv_pw_b"], np.float64)   # (L, D)
    # Fused FW' for layers 1..L-1: kh = f_w @ (pw @ y + pwb) = (f_w@pw) @ y + f_w@pwb
    fw_eff = fw.copy()
    kb = np.zeros((L, 2 * D), np.float64)
    for l in range(1, L):
        fw_eff[l] = fw[l] @ pw[l]
        kb[l] = fw[l] @ pwb[l]
    assert np.abs(kb).max() == 0.0, "nonzero conv_pw_b needs kb sign handling in gates"
    w = {
        "fwT": np.ascontiguousarray(np.transpose(fw_eff, (0, 2, 1))).astype(BF),
        "pwT": np.ascontiguousarray(np.transpose(np.asarray(inputs["conv_pw_w"], f), (0, 2, 1))).astype(BF),
        "w1T": np.ascontiguousarray(np.transpose(np.asarray(inputs["mlp_w1"], f), (0, 2, 1))).astype(BF),
        "w2T": np.ascontiguousarray(np.transpose(np.asarray(inputs["mlp_w2"], f), (0, 2, 1))).astype(BF),
        "dwK": np.ascontiguousarray(np.transpose(np.asarray(inputs["conv_dw_w"], f), (0, 2, 1))).astype(f),
        "dwb": np.asarray(inputs["conv_dw_b"], f),
        "pwb": np.asarray(inputs["conv_pw_b"], f),
        "kbv": kb.astype(f),
        "b1v": np.asarray(inputs["mlp_b1"], f),
        "b2v": np.asarray(inputs["mlp_b2"], f),
        "lng": np.concatenate([np.asarray(inputs["ln1_g"], f)[None], np.asarray(inputs["ln2_g"], f)], 0),
        "lnb": np.concatenate([np.asarray(inputs["ln1_b"], f)[None], np.asarray(inputs["ln2_b"], f)], 0),
    }
    x = np.asarray(inputs["x"], f)
    nb = x.shape[0]
    in_maps = []
    for b in range(nb):
        xTp = np.zeros((D, T + 3), BF)
        xTp[:, 3:] = x[b, :T].T.astype(BF)
        in_maps.append({"xT": xTp, **w})
    has_lnb = bool(np.any(w["lnb"] != 0.0))
    return in_maps, has_lnb


def kernel(**inputs):
    from concourse.bass_utils import run_bass_kernel_spmd

    T = int(np.asarray(inputs["x"]).shape[1])
    in_maps, has_lnb = make_host_inputs(inputs, T)
    nc = get_compiled_nc(T=T, has_lnb=has_lnb)
    res = run_bass_kernel_spmd(nc, in_maps, core_ids=list(range(len(in_maps))))
    out = np.stack([r["out"].T for r in res.results])
    return np.ascontiguousarray(out.astype(np.float32))


# revision 28
# speedup vs baseline: 1.0960x; 1.0076x over previous
"""Trainium2 Bass kernel for nn_BlockV2 (conv -> LN -> minGRU -> MLP x4).

Strategy: data-parallel over batch (B=8 -> 8 cores). Per core, activations
are kept in [D_partitions, T_free] layout and streamed through each layer in
chunks of 512 tokens; inter-layer activations ping-pong through DRAM.
The minGRU recurrence h_t = c_t*h_{t-1} + v_t runs on the VectorE
tensor_tensor_scan instruction (fp32 state), chained across chunks.

v2 changes vs baseline (1709us -> 1622us):
- LN statistics matmuls run in bf16 (the fp32 ones were LOW_HIGH two-pass,
  ~4x the cost / ~127us of TensorE). Precision is preserved by computing the
  stats on x' = x - 0.5 (the residual stream is a ~0.5 DC baseline with
  per-token channel variance down to ~1e-5; the DC shift is exact in fp32 and
  makes the E[x'^2]-mu'^2 cancellation benign). The mean/rstd broadcasts run
  on the idle GpSimd engine (partition_broadcast) instead of TensorE.
- f_w@conv_pw_w is fused host-side (FW'), so the mid-layer GRU kh matmul
  consumes the depthwise-conv output y directly - the pointwise conv output
  cv is only needed (in bf16, it is ~50x smaller than h) for the residual.
- The whole gate/scan path (z, cf, s, v, h) stays fp32 in small per-d-tile
  scratch tiles: the h signal is a ~5e-4 variation on 0.5 and bf16 gate
  storage destroys it (the final LN amplifies additive noise ~250x).
- conv_dw and the MLP output tile m are bf16; m only feeds conv_dw.
- layer-0 chunks (TensorE-light, VectorE-heavy) are interleaved into the
  first mid layer's chunk stream so TensorE never starves during the ramp.
"""
import sys

sys.path.insert(0, "/opt/trn_rl_repo")

from contextlib import ExitStack

import numpy as np
import ml_dtypes

import concourse.bass as bass
import concourse.tile as tile
from concourse import bacc, mybir

f32 = mybir.dt.float32
bf16 = mybir.dt.bfloat16
Alu = mybir.AluOpType
Act = mybir.ActivationFunctionType
BF = ml_dtypes.bfloat16

B, D, L, K, H = 8, 512, 4, 4, 2048
N_CORES = 8
LN_EPS = 1e-5
P = 128


def build_nc(T=4096, CH=512, has_lnb=False, debug_outs=False, use_gpsimd=False):
    NCH = T // CH
    DT = D // P      # 4 d-tiles
    HT = H // P      # 16 h-tiles
    E2 = 2 * D
    MT2 = E2 // P    # 8 m-tiles of the kh matmul

    nc = bacc.Bacc("TRN2", target_bir_lowering=False, debug=False)

    xT = nc.dram_tensor("xT", [D, T + 3], bf16, kind="ExternalInput")
    fwT = nc.dram_tensor("fwT", [L, D, E2], bf16, kind="ExternalInput")
    pwT = nc.dram_tensor("pwT", [L, D, D], bf16, kind="ExternalInput")
    w1T = nc.dram_tensor("w1T", [L, D, H], bf16, kind="ExternalInput")
    w2T = nc.dram_tensor("w2T", [L, H, D], bf16, kind="ExternalInput")
    dwK = nc.dram_tensor("dwK", [L, D, K], f32, kind="ExternalInput")
    dwb = nc.dram_tensor("dwb", [L, D], f32, kind="ExternalInput")
    pwb = nc.dram_tensor("pwb", [L, D], f32, kind="ExternalInput")
    kbv = nc.dram_tensor("kbv", [L, E2], f32, kind="ExternalInput")
    b1v = nc.dram_tensor("b1v", [L, H], f32, kind="ExternalInput")
    b2v = nc.dram_tensor("b2v", [L, D], f32, kind="ExternalInput")
    lng = nc.dram_tensor("lng", [L + 1, D], f32, kind="ExternalInput")
    lnb = nc.dram_tensor("lnb", [L + 1, D], f32, kind="ExternalInput")
    out_t = nc.dram_tensor("out", [D, T], f32, kind="ExternalOutput")
    if debug_outs:
        xs = [nc.dram_tensor(f"xs{i}", [D, T], f32, kind="ExternalOutput")
              for i in range(L)]
        dbg = {
            "mu": nc.dram_tensor("dbg_mu", [1, 512], f32, kind="ExternalOutput"),
            "var": nc.dram_tensor("dbg_var", [1, 512], f32, kind="ExternalOutput"),
            "rstd": nc.dram_tensor("dbg_rstd", [1, 512], f32, kind="ExternalOutput"),
            "a": nc.dram_tensor("dbg_a", [D, 512], f32, kind="ExternalOutput"),
        }
    else:
        dbg = None
        xs = [nc.dram_tensor(f"xs{i}", [D, T], f32) for i in range(2)]

    def dram3(tensor, c, width):
        return tensor.ap().rearrange("(dt p) t -> p dt t", p=P)[:, :, c * CH: c * CH + width]

    with tile.TileContext(nc) as tc, ExitStack() as ctx:
        sing = ctx.enter_context(tc.tile_pool(name="sing", bufs=1))
        wpool = ctx.enter_context(tc.tile_pool(name="w", bufs=1))
        wpool2 = ctx.enter_context(tc.tile_pool(name="w2", bufs=2))
        big = ctx.enter_context(tc.tile_pool(name="big", bufs=7))
        small = ctx.enter_context(tc.tile_pool(name="small", bufs=8))
        mp = ctx.enter_context(tc.tile_pool(name="mp", bufs=2))
        hidp = ctx.enter_context(tc.tile_pool(name="hid", bufs=1))
        statp = ctx.enter_context(tc.tile_pool(name="stat", bufs=5))
        hcp = ctx.enter_context(tc.tile_pool(name="hc", bufs=2))
        zsp = ctx.enter_context(tc.tile_pool(name="zs", bufs=8))
        bcp = ctx.enter_context(tc.tile_pool(name="bc", bufs=2))
        psmm = ctx.enter_context(tc.tile_pool(name="psmm", bufs=6, space="PSUM"))
        psst = ctx.enter_context(tc.tile_pool(name="psst", bufs=2, space="PSUM"))


        ones_col = sing.tile([P, 1], bf16)
        nc.vector.memset(ones_col, 1.0)
        ones_row = sing.tile([1, P], bf16)
        nc.vector.memset(ones_row, 1.0)
        eps1 = sing.tile([1, 1], f32)
        nc.vector.memset(eps1, LN_EPS)
        zero1 = sing.tile([1, 1], f32)
        nc.vector.memset(zero1, 0.0)
        nhalf_col = sing.tile([P, 1], f32)
        nc.vector.memset(nhalf_col, -0.5)
        zero_col = sing.tile([P, 1], f32)
        nc.vector.memset(zero_col, 0.0)
        dw_sb = sing.tile([P, L * DT, K], f32)
        nc.sync.dma_start(out=dw_sb, in_=dwK.ap().rearrange("l (dt p) k -> p (l dt) k", p=P))
        dwb_sb = sing.tile([P, L * DT], f32)
        nc.sync.dma_start(out=dwb_sb, in_=dwb.ap().rearrange("l (dt p) -> p (l dt)", p=P))
        pwb_sb = sing.tile([P, L * DT], f32)
        nc.sync.dma_start(out=pwb_sb, in_=pwb.ap().rearrange("l (dt p) -> p (l dt)", p=P))
        kb_sb = sing.tile([P, L * MT2], f32)
        nc.sync.dma_start(out=kb_sb, in_=kbv.ap().rearrange("l (mt p) -> p (l mt)", p=P))
        b1_sb = sing.tile([P, L * HT], f32)
        nc.sync.dma_start(out=b1_sb, in_=b1v.ap().rearrange("l (ht p) -> p (l ht)", p=P))
        b2_sb = sing.tile([P, L * DT], f32)
        nc.sync.dma_start(out=b2_sb, in_=b2v.ap().rearrange("l (dt p) -> p (l dt)", p=P))
        lng_sb = sing.tile([P, (L + 1) * DT], f32)
        nc.sync.dma_start(out=lng_sb, in_=lng.ap().rearrange("l (dt p) -> p (l dt)", p=P))
        lnb_sb = sing.tile([P, (L + 1) * DT], f32)
        nc.sync.dma_start(out=lnb_sb, in_=lnb.ap().rearrange("l (dt p) -> p (l dt)", p=P))

        def load_w(kind, dram, l, shape, pool=wpool):
            t = pool.tile(shape, bf16, tag=kind, name=f"{kind}{l}")
            nc.sync.dma_start(out=t, in_=dram.ap()[l].rearrange("(kt p) e -> p kt e", p=P))
            return t


        # ---------- LN pieces ----------
        # The residual-stream LN inputs sit on a ~0.5 DC baseline with tiny
        # per-token variance (down to ~1e-5 at the last layer), so the stats
        # are computed on x' = x - shift (exact in fp32, then bf16): the
        # E[x'^2] - mu'^2 cancellation is benign once the DC is removed.
        def ln_stats(x_tile, shift):
            """shifted bf16 copy + xsq + S'/Q' bf16 matmuls. Returns psum tiles."""
            x_bf = small.tile([P, DT, CH], bf16, tag="small", name="x_bf")
            for d in range(DT):
                if d % 2 == 0:
                    sb = nhalf_col if shift == 0.5 else zero_col
                    nc.scalar.activation(out=x_bf[:, d, :], in_=x_tile[:, d, :],
                                         func=Act.Identity, bias=sb[:, :], scale=1.0)
                else:
                    nc.vector.tensor_scalar(
                        out=x_bf[:, d, :], in0=x_tile[:, d, :],
                        scalar1=-shift, scalar2=None, op0=Alu.add)
            xsq = small.tile([P, DT, CH], bf16, tag="small", name="xsq")
            for d in range(DT):
                nc.vector.tensor_mul(xsq[:, d, :], x_bf[:, d, :], x_bf[:, d, :])
            S_ps = psst.tile([1, CH], f32, tag="ps_stat", name="S_ps")
            for kt in range(DT):
                nc.tensor.matmul(S_ps[:, :], ones_col[:, :], x_bf[:, kt, :],
                                 start=(kt == 0), stop=(kt == DT - 1))
            Q_ps = psst.tile([1, CH], f32, tag="ps_stat", name="Q_ps")
            for kt in range(DT):
                nc.tensor.matmul(Q_ps[:, :], ones_col[:, :], xsq[:, kt, :],
                                 start=(kt == 0), stop=(kt == DT - 1))
            return S_ps, Q_ps

        def ln_apply(x_tile, S_ps, Q_ps, slot, out_bf16, shift, dump=None):
            """stat algebra + gpsimd broadcasts + center-in-place + scale.
            mu' = mean(x - shift); center computes (x - shift) - mu'."""
            mu = statp.tile([1, CH], f32, tag="stat", name="mu")
            nc.scalar.activation(out=mu[:, :], in_=S_ps[:, :], func=Act.Identity,
                                 bias=0.0, scale=1.0 / D)
            musq = statp.tile([1, CH], f32, tag="stat", name="musq")
            nc.scalar.activation(out=musq[:, :], in_=mu[:, :], func=Act.Square,
                                 bias=0.0, scale=1.0)
            var = statp.tile([1, CH], f32, tag="stat", name="var")
            nc.vector.scalar_tensor_tensor(
                var[:, :], Q_ps[:, :], 1.0 / D, musq[:, :], Alu.mult, Alu.subtract)
            lnv = statp.tile([1, CH], f32, tag="stat", name="lnv")
            nc.scalar.activation(out=lnv[:, :], in_=var[:, :], func=Act.Ln,
                                 bias=eps1[:, :], scale=1.0)
            rstd = statp.tile([1, CH], f32, tag="stat", name="rstd")
            nc.scalar.activation(out=rstd[:, :], in_=lnv[:, :], func=Act.Exp, scale=-0.5)
            bc_mu = bcp.tile([P, CH], f32, tag="bc", name="bc_mu")
            nc.gpsimd.partition_broadcast(bc_mu[:, :], mu[:, :])
            bc_r = bcp.tile([P, CH], f32, tag="bc", name="bc_r")
            nc.gpsimd.partition_broadcast(bc_r[:, :], rstd[:, :])
            for d in range(DT):
                nc.vector.scalar_tensor_tensor(
                    x_tile[:, d, :], x_tile[:, d, :], -shift, bc_mu[:, :],
                    Alu.add, Alu.subtract)
            if out_bf16:
                a_t = small.tile([P, DT, CH], bf16, tag="small", name="a_t")
            else:
                a_t = big.tile([P, DT, CH], f32, tag="big", name="a_t")
            for d in range(DT):
                nc.vector.scalar_tensor_tensor(
                    a_t[:, d, :], x_tile[:, d, :], lng_sb[:, slot * DT + d: slot * DT + d + 1],
                    bc_r[:, :], Alu.mult, Alu.mult)
            if dump is not None:
                nc.sync.dma_start(out=dump["mu"].ap(), in_=mu)
                nc.sync.dma_start(out=dump["var"].ap(), in_=var)
                nc.sync.dma_start(out=dump["rstd"].ap(), in_=rstd)
                a_f32 = big.tile([P, DT, CH], f32, tag="big", name="a_f32")
                for d in range(DT):
                    nc.vector.tensor_copy(out=a_f32[:, d, :], in_=a_t[:, d, :])
                nc.sync.dma_start(out=dump["a"].ap().rearrange("(dt p) t -> p dt t", p=P), in_=a_f32)
            if has_lnb:
                for d in range(DT):
                    nc.vector.tensor_scalar(
                        out=a_t[:, d, :], in0=a_t[:, d, :],
                        scalar1=lnb_sb[:, slot * DT + d: slot * DT + d + 1], scalar2=None,
                        op0=Alu.add)
            return a_t

        def mlp_chunk(a_t, l, w1_sb, w2_sb, out_tile, out_off, out_f32_scalar_evac):
            hid = hidp.tile([P, HT, CH], bf16, tag="hid", name="hid")
            for mt in range(HT):
                ps = psmm.tile([P, CH], f32, tag="mm", name="ps1")
                for kt in range(DT):
                    nc.tensor.matmul(ps[:, :], w1_sb[:, kt, bass.ts(mt, P)], a_t[:, kt, :],
                                     start=(kt == 0), stop=(kt == DT - 1))
                RELU_SPLIT = False
                if RELU_SPLIT and mt % 2 == 1:
                    nc.vector.tensor_scalar(
                        out=hid[:, mt, :], in0=ps[:, :],
                        scalar1=b1_sb[:, l * HT + mt: l * HT + mt + 1], scalar2=0.0,
                        op0=Alu.add, op1=Alu.max)
                else:
                    nc.scalar.activation(out=hid[:, mt, :], in_=ps[:, :], func=Act.Relu,
                                         bias=b1_sb[:, l * HT + mt: l * HT + mt + 1], scale=1.0)
            for mt in range(DT):
                ps = psmm.tile([P, CH], f32, tag="mm", name="ps2")
                for kt in range(HT):
                    nc.tensor.matmul(ps[:, :], w2_sb[:, kt, bass.ts(mt, P)], hid[:, kt, :],
                                     start=(kt == 0), stop=(kt == HT - 1))
                if out_f32_scalar_evac:
                    nc.scalar.activation(out=out_tile[:, mt, out_off: out_off + CH], in_=ps[:, :],
                                         func=Act.Identity,
                                         bias=b2_sb[:, l * DT + mt: l * DT + mt + 1], scale=1.0)
                else:
                    nc.vector.tensor_scalar(
                        out=out_tile[:, mt, out_off: out_off + CH], in0=ps[:, :],
                        scalar1=b2_sb[:, l * DT + mt: l * DT + mt + 1], scalar2=None,
                        op0=Alu.add)

        def conv_dw(m_t, l):
            """depthwise conv over bf16 input tile [P, DT, CH+3] -> y bf16."""
            acc = small.tile([P, DT, CH], bf16, tag="small", name="acc")
            y = small.tile([P, DT, CH], bf16, tag="small", name="y")
            for d in range(DT):
                # first tap on ScalarE (idle in the Vector-jammed windows):
                # acc = m0 * w0 + dwb via Identity with per-partition scale AP
                nc.scalar.activation(
                    out=acc[:, d, :], in_=m_t[:, d, 0: CH], func=Act.Identity,
                    bias=dwb_sb[:, l * DT + d: l * DT + d + 1],
                    scale=dw_sb[:, l * DT + d, 0:1])
                for j in range(1, K - 1):
                    nc.vector.scalar_tensor_tensor(
                        acc[:, d, :], m_t[:, d, j: j + CH], dw_sb[:, l * DT + d, j: j + 1],
                        acc[:, d, :], Alu.mult, Alu.add)
                nc.vector.scalar_tensor_tensor(
                    y[:, d, :], m_t[:, d, K - 1: K - 1 + CH], dw_sb[:, l * DT + d, K - 1: K],
                    acc[:, d, :], Alu.mult, Alu.add)
            return y

        def conv_pw(y, l, pw_sb):
            """pointwise conv -> cv_bf (bf16 only; feeds the residual add)."""
            cv_bf = small.tile([P, DT, CH], bf16, tag="small", name="cv_bf")
            for mt in range(DT):
                ps = psmm.tile([P, CH], f32, tag="mm", name="ps3")
                for kt in range(DT):
                    nc.tensor.matmul(ps[:, :], pw_sb[:, kt, bass.ts(mt, P)], y[:, kt, :],
                                     start=(kt == 0), stop=(kt == DT - 1))
                nc.scalar.activation(out=cv_bf[:, mt, :], in_=ps[:, :], func=Act.Identity,
                                     bias=pwb_sb[:, l * DT + mt: l * DT + mt + 1], scale=1.0)
            return cv_bf

        def conv_pw_f32(y, l, pw_sb):
            """pointwise conv -> cv fp32 (layer 0: feeds LN directly)."""
            cv = big.tile([P, DT, CH], f32, tag="big", name="cv")
            for mt in range(DT):
                ps = psmm.tile([P, CH], f32, tag="mm", name="ps3")
                for kt in range(DT):
                    nc.tensor.matmul(ps[:, :], pw_sb[:, kt, bass.ts(mt, P)], y[:, kt, :],
                                     start=(kt == 0), stop=(kt == DT - 1))
                nc.scalar.activation(out=cv[:, mt, :], in_=ps[:, :], func=Act.Identity,
                                     bias=pwb_sb[:, l * DT + mt: l * DT + mt + 1], scale=1.0)
            return cv

        def gru_scan(rhs_bf, fw_sb, l, h_prev):
            """kh matmuls + gates + scan, per-d-tile pipelined. All gate/scan
            values fp32 (the h signal is a ~5e-4 variation on a 0.5 baseline;
            bf16 gate storage destroys it). Returns (hcol, h)."""
            h = big.tile([P, DT, CH], f32, tag="big", name="h")
            hcol = hcp.tile([P, DT, 1], f32, tag="hc", name="hcol")
            for d in range(DT):
                ps_k = psmm.tile([P, CH], f32, tag="mm", name="ps_k")
                for kt in range(DT):
                    nc.tensor.matmul(ps_k[:, :], fw_sb[:, kt, bass.ts(d, P)],
                                     rhs_bf[:, kt, :], start=(kt == 0), stop=(kt == DT - 1))
                ps_h = psmm.tile([P, CH], f32, tag="mm", name="ps_h")
                for kt in range(DT):
                    nc.tensor.matmul(ps_h[:, :], fw_sb[:, kt, bass.ts(DT + d, P)],
                                     rhs_bf[:, kt, :], start=(kt == 0), stop=(kt == DT - 1))
                kb_k = kb_sb[:, l * MT2 + d: l * MT2 + d + 1]
                kb_h = kb_sb[:, l * MT2 + DT + d: l * MT2 + DT + d + 1]
                z = zsp.tile([P, CH], f32, tag="zs", name="z")
                nc.scalar.activation(out=z[:, :], in_=ps_k[:, :], func=Act.Sigmoid,
                                     bias=kb_k, scale=1.0)
                cf = zsp.tile([P, CH], f32, tag="zs", name="cf")
                nc.scalar.activation(out=cf[:, :], in_=ps_k[:, :], func=Act.Sigmoid,
                                     bias=kb_k, scale=-1.0)
                s = zsp.tile([P, CH], f32, tag="zs", name="s")
                nc.scalar.activation(out=s[:, :], in_=ps_h[:, :], func=Act.Sigmoid,
                                     bias=kb_h, scale=1.0)
                nc.vector.scalar_tensor_tensor(
                    s[:, :], ps_h[:, :], 0.5, s[:, :], Alu.add, Alu.max)
                v = zsp.tile([P, CH], f32, tag="zs", name="v")
                nc.vector.tensor_mul(v[:, :], z[:, :], s[:, :])
                init = 0.5 if h_prev is None else h_prev[:, d, 0:1]
                nc.vector.tensor_tensor_scan(h[:, d, :], cf[:, :], v[:, :], init,
                                             Alu.mult, Alu.add)
            nc.vector.tensor_copy(out=hcol, in_=h[:, :, CH - 1: CH])
            return hcol, h

        def gru_res(h, base):
            """res = h + base, in place into h."""
            for d in range(DT):
                nc.vector.tensor_add(h[:, d, :], h[:, d, :], base[:, d, :])
            return h

        # wait-for-scale note on gates: scale=-1 sigmoid bias sign
        # cf = sigmoid(-(k + kb)) requires bias applied before negation:
        # activation computes func(scale*x + bias) so cf uses bias=-kb? see
        # host: we pass kb and use scale=-1 -> sigmoid(-k + kb) WRONG unless
        # kb == 0. Handled host-side: kbias is folded only when nonzero is
        # impossible (conv_pw_b zeros); assert there.

        # ---------- chunk programs ----------
        chunks = []
        wd0 = {}
        st0 = {"h": None}

        def mk_l0(c):
            def s0(_):
                if c == 0:
                    wd0["fw"] = load_w("fw", fwT, 0, [P, DT, E2], wpool2)
                    wd0["pw"] = load_w("pw", pwT, 0, [P, DT, D], wpool2)
                if c == 1:
                    wd0["w1"] = load_w("w1", w1T, 0, [P, DT, H])
                    wd0["w2"] = load_w("w2", w2T, 0, [P, HT, D])
                x_in = small.tile([P, DT, CH + 3], bf16, tag="small", name="x_in")
                nc.sync.dma_start(out=x_in, in_=xT.ap().rearrange("(dt p) t -> p dt t", p=P)[:, :, c * CH: c * CH + CH + 3])
                return conv_dw(x_in, 0)

            def s1(y):
                cv = conv_pw_f32(y, 0, wd0["pw"])
                S_ps, Q_ps = ln_stats(cv, shift=0.0)
                return cv, S_ps, Q_ps

            def s2(art):
                cv, S_ps, Q_ps = art
                n = ln_apply(cv, S_ps, Q_ps, 0, out_bf16=False, shift=0.0)
                n_bf = small.tile([P, DT, CH], bf16, tag="small", name="n_bf")
                for d in range(DT):
                    nc.gpsimd.tensor_copy(out=n_bf[:, d, :], in_=n[:, d, :])
                return n, n_bf

            def s3(art):
                n, n_bf = art
                st0["h"], h = gru_scan(n_bf, wd0["fw"], 0, st0["h"])
                res = gru_res(h, n)
                nc.sync.dma_start(out=dram3(xs[0], c, CH), in_=res)

            return [s0, s1, s2, s3]

        wdm = [{} for _ in range(L - 1)]
        stm = [{"h": None, "m_prev": None} for _ in range(L - 1)]

        def mk_mid(i, c):
            wd, st = wdm[i], stm[i]
            src_d, dst_d = (xs[i], xs[i + 1]) if len(xs) == L else (xs[i % 2], xs[(i + 1) % 2])
            c_w12 = 1 if i == 0 else 2
            c_fwpw = 1 if i == 0 else 3

            def s0(_):
                if i > 0 and c == c_w12:
                    wd["w1"] = load_w("w1", w1T, i, [P, DT, H])
                    wd["w2"] = load_w("w2", w2T, i, [P, HT, D])
                if c == c_fwpw:
                    wd["fw"] = load_w("fw", fwT, i + 1, [P, DT, E2], wpool2)
                    wd["pw"] = load_w("pw", pwT, i + 1, [P, DT, D], wpool2)
                x_in = big.tile([P, DT, CH], f32, tag="big", name="x_in")
                nc.sync.dma_start(out=x_in, in_=dram3(src_d, c, CH))
                S_ps, Q_ps = ln_stats(x_in, shift=0.5)
                return x_in, S_ps, Q_ps

            def s1(art):
                x_in, S_ps, Q_ps = art
                return ln_apply(x_in, S_ps, Q_ps, 1 + i, out_bf16=True, shift=0.5)

            def s2(a):
                if i == 0:
                    w1_sb, w2_sb = wd0["w1"], wd0["w2"]
                else:
                    w1_sb, w2_sb = wd["w1"], wd["w2"]
                m = mp.tile([P, DT, CH + 3], bf16, tag="m_bf", name="m")
                mlp_chunk(a, i, w1_sb, w2_sb, m, 3, out_f32_scalar_evac=False)
                if c == 0:
                    nc.vector.memset(m[:, :, 0:3], 0.0)
                else:
                    nc.vector.tensor_copy(out=m[:, :, 0:3], in_=st["m_prev"][:, :, CH: CH + 3])
                st["m_prev"] = m
                return m

            def s3(m):
                y = conv_dw(m, i + 1)
                st["h"], h = gru_scan(y, wd["fw"], i + 1, st["h"])
                return y, h

            def s4(art):
                y, h = art
                cv_bf = conv_pw(y, i + 1, wd["pw"])
                res = gru_res(h, cv_bf)
                nc.sync.dma_start(out=dram3(dst_d, c, CH), in_=res)

            return [s0, s1, s2, s3, s4]

        wdt = {}
        src_t = xs[L - 1] if len(xs) == L else xs[(L - 1) % 2]

        def mk_tail(c):
            def s0(_):
                if c == 2:
                    wdt["w1"] = load_w("w1", w1T, L - 1, [P, DT, H])
                    wdt["w2"] = load_w("w2", w2T, L - 1, [P, HT, D])
                x_in = big.tile([P, DT, CH], f32, tag="big", name="x_in")
                nc.sync.dma_start(out=x_in, in_=dram3(src_t, c, CH))
                S_ps, Q_ps = ln_stats(x_in, shift=0.5)
                return x_in, S_ps, Q_ps

            def s1(art):
                x_in, S_ps, Q_ps = art
                return ln_apply(x_in, S_ps, Q_ps, L, out_bf16=True, shift=0.5,
                                dump=(dbg if (debug_outs and c == 0) else None))

            def s2(a):
                o = big.tile([P, DT, CH], f32, tag="big", name="o")
                mlp_chunk(a, L - 1, wdt["w1"], wdt["w2"], o, 0, out_f32_scalar_evac=True)
                nc.sync.dma_start(out=dram3(out_t, c, CH), in_=o)

            return [s0, s1, s2]

        # ---------- emission: interleave L0 into M0, then serial ----------
        # L0c0..L0c3 M0c0 L0c4 M0c1 L0c5 ... L0c7 M0c4..M0c7 M1c0..7 ...
        # M0's chunk c load must land strictly after L0's chunk-c store
        # (pos(M0c) >= pos(L0c)+4 keeps the DRAM RAW ordering).
        INTERLEAVE = True
        if INTERLEAVE:
            for c in range(4):
                chunks.append(mk_l0(c))
            for c in range(NCH - 4):
                chunks.append(mk_mid(0, c))
                chunks.append(mk_l0(c + 4))
            for c in range(NCH - 4, NCH):
                chunks.append(mk_mid(0, c))
        else:
            for c in range(NCH):
                chunks.append(mk_l0(c))
            for c in range(NCH):
                chunks.append(mk_mid(0, c))
        for i in range(1, L - 1):
            for c in range(NCH):
                chunks.append(mk_mid(i, c))
        for c in range(NCH):
            chunks.append(mk_tail(c))

        NST = 5
        arts = [None] * len(chunks)
        for g in range(len(chunks) + NST - 1):
            for k in range(NST):
                idx = g - k
                if 0 <= idx < len(chunks) and k < len(chunks[idx]):
                    arts[idx] = chunks[idx][k](arts[idx])

    return nc


_CACHE = {}


def get_compiled_nc(T=4096, CH=512, has_lnb=False, **kw):
    key = (T, CH, has_lnb, tuple(sorted(kw.items())))
    if key not in _CACHE:
        nc = build_nc(T, CH, has_lnb, **kw)
        nc.compile()
        _CACHE[key] = nc
    return _CACHE[key]


def _make_dw_diag(dw_w):
    """dw_w (L, K, D) -> diagonal-tap lhsT tiles (L, 128, DT*K*128) bf16.
    arr[l, k, dt, j, c] = (k==c) * dw_w[l, j, dt*128+c]."""
    DT = D // P
    arr = np.zeros((L, P, DT, K, P), np.float32)
    idx = np.arange(P)
    for l in range(L):
        for dt in range(DT):
            for j in range(K):
                arr[l, idx, dt, j, idx] = dw_w[l, j, dt * P + idx]
    return arr.reshape(L, P, DT * K * P).astype(BF)


def make_host_inputs(inputs, T=4096):
    f = np.float32
    fw = np.asarray(inputs["f_w"], np.float64)          # (L, 2D, D)
    pw = np.asarray(inputs["conv_pw_w"], np.float64)    # (L, D, D)
    pwb = np.asarray(inputs["conv_pw_b"], np.float64)   # (L, D)
    # Fused FW' for layers 1..L-1: kh = f_w @ (pw @ y + pwb) = (f_w@pw) @ y + f_w@pwb
    fw_eff = fw.copy()
    kb = np.zeros((L, 2 * D), np.float64)
    for l in range(1, L):
        fw_eff[l] = fw[l] @ pw[l]
        kb[l] = fw[l] @ pwb[l]
    assert np.abs(kb).max() == 0.0, "nonzero conv_pw_b needs kb sign handling in gates"
    w = {
        "fwT": np.ascontiguousarray(np.transpose(fw_eff, (0, 2, 1))).astype(BF),
        "pwT": np.ascontiguousarray(np.transpose(np.asarray(inputs["conv_pw_w"], f), (0, 2, 1))).astype(BF),
        "w1T": np.ascontiguousarray(np.transpose(np.asarray(inputs["mlp_w1"], f), (0, 2, 1))).astype(BF),
        "w2T": np.ascontiguousarray(np.transpose(np.asarray(inputs["mlp_w2"], f), (0, 2, 1))).astype(BF),
        "dwK": np.ascontiguousarray(np.transpose(np.asarray(inputs["conv_dw_w"], f), (0, 2, 1))).astype(f),
        "dwb": np.asarray(inputs["conv_dw_b"], f),
        "pwb": np.asarray(inputs["conv_pw_b"], f),
        "kbv": kb.astype(f),
        "b1v": np.asarray(inputs["mlp_b1"], f),
        "b2v": np.asarray(inputs["mlp_b2"], f),
        "lng": np.concatenate([np.asarray(inputs["ln1_g"], f)[None], np.asarray(inputs["ln2_g"], f)], 0),
        "lnb": np.concatenate([np.asarray(inputs["ln1_b"], f)[None], np.asarray(inputs["ln2_b"], f)], 0),
    }
    x = np.asarray(inputs["x"], f)
    nb = x.shape[0]
    in_maps = []
    for b in range(nb):
        xTp = np.zeros((D, T + 3), BF)
        xTp[:, 3:] = x[b, :T].T.astype(BF)
        in_maps.append({"xT": xTp, **w})
    has_lnb = bool(np.any(w["lnb"] != 0.0))
    return in_maps, has_lnb


def kernel(**inputs):
    from concourse.bass_utils import run_bass_kernel_spmd

    T = int(np.asarray(inputs["x"]).shape[1])
    in_maps, has_lnb = make_host_inputs(inputs, T)
    nc = get_compiled_nc(T=T, has_lnb=has_lnb)
    res = run_bass_kernel_spmd(nc, in_maps, core_ids=list(range(len(in_maps))))
    out = np.stack([r["out"].T for r in res.results])
    return np.ascontiguousarray(out.astype(np.float32))
